# revision 13
# baseline (speedup 1.0000x reference)
"""Trainium2 Bass kernel for nn_BILEAT_62328565400310 (sparse_attention).

Self-contained: takes FULL inputs (as from setup_inputs()), shards batch B=32
across 8 NeuronCores (4 examples/core, weights replicated), runs one fused
Bass/Tile kernel per core, gathers outputs, computes the (tiny) margin-loss
reduction on host from device-computed logits.

Device math per example (derived in transposed layouts so every matmul is a
native lhsT.T @ rhs with the contraction on partitions; float32r = fp32 data
processed at FP22 by the PE at bf16 speed):
  h_aeT[d,s], h_opT[d,s]  = w.T @ x.T           (+ per-partition bias)
  o_ae[s,3]   = relu(h_aeT).T @ w_ae            (3-class softmax -> p_ae[s])
  GT[e,s]     = W.T-as-lhsT @ h_aeT
  A[s,t]      = GT.T @ h_opT                    (bilinear scores)
  F[s,t]      = exp((A*p_ae[s]) * dist[s,t])    (dist has zero diag ->
                F diag = 1, corrected in colsum & value matmul)
  den_ae[t]   = colsum(F) - 1                   (ones-matmul)
  ae_primeT   = (h_ae-as-lhsT @ F - h_aeT) * (1/den_ae broadcast)
  (op path identical on A.T via PE transpose, p_op, h_op)
  o_prime[s,7] = [h_aeT; ae_primeT; h_opT; op_primeT]-as-lhsT @ w_fc + b_fc
"""
import numpy as np

B, S_FULL, H, D2 = 32, 1024, 768, 768
NN, NL = 3, 7
NCORES = 8
EPC = B // NCORES          # examples per core
KH = H // 128              # 6 h-tiles
KD = D2 // 128             # 6 d-tiles
C0 = 896                   # toeplitz column offset (= 128*(S/128 - 1))


def _build_module(n_ex=EPC, S=S_FULL):
    import concourse.bacc as bacc
    import concourse.mybir as mybir
    import concourse.tile as tile

    f32 = mybir.dt.float32
    f32r = mybir.dt.float32r
    Alu = mybir.AluOpType
    Act = mybir.ActivationFunctionType
    Ax = mybir.AxisListType

    NT = S // 128            # s/t tiles
    CH = [(o, min(512, S - o)) for o in range(0, S, 512)]  # free-dim chunks
    W2 = C0 + S              # toeplitz buffer width (dev configs use same C0)

    nc = bacc.Bacc(None, target_bir_lowering=False)

    # ---- DRAM I/O ----
    xt_d = nc.dram_tensor("xt", [n_ex, H, S], f32r, kind="ExternalInput")
    wae1_d = nc.dram_tensor("wae1", [H, D2], f32r, kind="ExternalInput")
    wop1_d = nc.dram_tensor("wop1", [H, D2], f32r, kind="ExternalInput")
    wbil_d = nc.dram_tensor("wbil", [D2, D2], f32r, kind="ExternalInput")
    wfc_d = nc.dram_tensor("wfc", [4 * D2, NL], f32r, kind="ExternalInput")
    wae3_d = nc.dram_tensor("wae3", [D2, NN], f32, kind="ExternalInput")
    wop3_d = nc.dram_tensor("wop3", [D2, NN], f32, kind="ExternalInput")
    bae1_d = nc.dram_tensor("bae1", [128, KH], f32, kind="ExternalInput")
    bop1_d = nc.dram_tensor("bop1", [128, KH], f32, kind="ExternalInput")
    bae3_d = nc.dram_tensor("bae3", [128, NN], f32, kind="ExternalInput")
    bop3_d = nc.dram_tensor("bop3", [128, NN], f32, kind="ExternalInput")
    bfc_d = nc.dram_tensor("bfc", [128, NL], f32, kind="ExternalInput")
    dist_d = nc.dram_tensor("dist", [128, W2], f32, kind="ExternalInput")
    ident_d = nc.dram_tensor("ident", [128, 128], f32r, kind="ExternalInput")
    ones_d = nc.dram_tensor("ones", [128, 128], f32, kind="ExternalInput")
    oprime_d = nc.dram_tensor("o_prime", [n_ex, S, NL], f32, kind="ExternalOutput")
    oae_d = nc.dram_tensor("o_ae", [n_ex, S, NN], f32, kind="ExternalOutput")
    oop_d = nc.dram_tensor("o_op", [n_ex, S, NN], f32, kind="ExternalOutput")

    with tile.TileContext(nc) as tc:
        with (
            tc.tile_pool(name="cst", bufs=1) as cst,
            tc.tile_pool(name="sb", bufs=1) as sb,
            tc.tile_pool(name="ps", bufs=1, space="PSUM") as ps,
        ):
            # ---- persistent constants ----
            dist = cst.tile([128, W2], f32, name="dist_sb")
            nc.sync.dma_start(dist[:], dist_d[:])
            ident = cst.tile([128, 128], f32r, name="ident_sb")
            nc.sync.dma_start(ident[:], ident_d[:])
            ones = cst.tile([128, 128], f32, name="ones_sb")
            nc.sync.dma_start(ones[:], ones_d[:])
            NLP = NL + 1  # pad to even for fp32r lhsT
            wfc = cst.tile([128, 4 * KD * NLP], f32r, name="wfc_sb")
            nc.vector.memset(wfc[:].bitcast(f32), 0.0)
            for kk in range(4 * KD):
                nc.sync.dma_start(wfc[:, kk * NLP: kk * NLP + NL],
                                  wfc_d[kk * 128:(kk + 1) * 128, :])
            wae3 = cst.tile([128, KD * NN], f32, name="wae3_sb")
            wop3 = cst.tile([128, KD * NN], f32, name="wop3_sb")
            for kk in range(KD):
                nc.sync.dma_start(wae3[:, kk * NN:(kk + 1) * NN],
                                  wae3_d[kk * 128:(kk + 1) * 128, :])
                nc.sync.dma_start(wop3[:, kk * NN:(kk + 1) * NN],
                                  wop3_d[kk * 128:(kk + 1) * 128, :])
            bae1 = cst.tile([128, KH], f32, name="bae1_sb")
            nc.sync.dma_start(bae1[:], bae1_d[:])
            bop1 = cst.tile([128, KH], f32, name="bop1_sb")
            nc.sync.dma_start(bop1[:], bop1_d[:])
            bae3 = cst.tile([128, NN], f32, name="bae3_sb")
            nc.sync.dma_start(bae3[:], bae3_d[:])
            bop3 = cst.tile([128, NN], f32, name="bop3_sb")
            nc.sync.dma_start(bop3[:], bop3_d[:])
            bfc = cst.tile([128, NL], f32, name="bfc_sb")
            nc.sync.dma_start(bfc[:], bfc_d[:])

            def dslice(t):
                # dist rows for s-tile t: toeplitz slice (diag term is 0)
                return dist[:, C0 - 128 * t: C0 - 128 * t + S]

            for e in range(n_ex):
                # =============== P0: load x, weights; projections ===============
                xk = []
                for k in range(KH):
                    t = sb.tile([128, S], f32r, name=f"x_{e}_{k}", tag="tagX", bufs=8)
                    nc.sync.dma_start(t[:], xt_d[e, k * 128:(k + 1) * 128, :])
                    xk.append(t)
                wae1 = []
                wop1 = []
                for k in range(KH):
                    t = sb.tile([128, D2], f32r, name=f"wae1_{e}_{k}", tag="tagW", bufs=12)
                    nc.sync.dma_start(t[:], wae1_d[k * 128:(k + 1) * 128, :])
                    wae1.append(t)
                for k in range(KH):
                    t = sb.tile([128, D2], f32r, name=f"wop1_{e}_{k}", tag="tagW", bufs=12)
                    nc.sync.dma_start(t[:], wop1_d[k * 128:(k + 1) * 128, :])
                    wop1.append(t)

                o3_ps = ps.tile([128, 2 * NT * NN * KD], f32, name=f"o3_ps_{e}", tag="ps3", bufs=1)
                o3_v = o3_ps.rearrange("p (r c k) -> p r c k", c=NN, k=KD)
                # transposed FC accumulator: o_primeT [NLP, S]; one accumulation
                # group per 512-chunk bank spanning all 4 parts x 6 dt
                fcT_ps = ps.tile([NLP, S], f32, name=f"fcT_{e}", tag="psFC", bufs=1)

                def fc_mm(part, dt, rhs_tile):
                    for o, w in CH:
                        nc.tensor.matmul(
                            fcT_ps[:, o:o + w],
                            wfc[:, (part * KD + dt) * NLP:(part * KD + dt + 1) * NLP],
                            rhs_tile[:, o:o + w],
                            start=(part == 0 and dt == 0),
                            stop=(part == 1 and dt == KD - 1))

                haeT, hopT = [], []
                for (wsb, bvec, o3off, fcoff, hlist) in (
                    (wae1, bae1, 0, 0, haeT),
                    (wop1, bop1, NT * NN, 2 * KD, hopT),
                ):
                    for dt in range(KD):
                        mm = ps.tile([128, S], f32, name=f"proj_{e}_{dt}", tag="psMM", bufs=2)
                        for o, w in CH:
                            for k in range(KH):
                                nc.tensor.matmul(
                                    mm[:, o:o + w],
                                    wsb[k][:, dt * 128:(dt + 1) * 128],
                                    xk[k][:, o:o + w],
                                    start=(k == 0), stop=(k == KH - 1))
                        hT = sb.tile([128, S], f32r, name=f"hT_{e}_{dt}", tag="tagHT", bufs=12)
                        nc.scalar.activation(hT[:], mm[:], Act.Identity,
                                             bias=bvec[:, dt:dt + 1])
                        hlist.append(hT)
                        fc_mm(0 if fcoff == 0 else 2, dt, hT)
                        # relu copy -> o_ae/o_op logits (w_ae path then w_op path)
                        rl = sb.tile([128, S], f32r, name=f"rl_{e}_{dt}", tag="tagTMP", bufs=3)
                        nc.scalar.activation(rl[:], mm[:], Act.Relu,
                                             bias=bvec[:, dt:dt + 1])
                        w3 = wae3 if o3off == 0 else wop3
                        for st in range(NT):
                            nc.tensor.matmul(
                                o3_v[:, (0 if o3off == 0 else NT) + st, :, dt],
                                rl[:, st * 128:(st + 1) * 128].bitcast(f32),
                                w3[:, dt * NN:(dt + 1) * NN],
                                start=True, stop=True)

                # ---- 3-class softmaxes -> p_ae, p_op; emit o_ae/o_op ----
                o3pre = sb.tile([128, 2 * NT * NN], f32, name=f"o3pre_{e}", tag="st_o3pre", bufs=2)
                nc.vector.reduce_sum(o3pre[:], o3_v, axis=Ax.X)
                pvec = {}
                for name, o3off, b3, od in (("ae", 0, bae3, oae_d), ("op", NT * NN, bop3, oop_d)):
                    o3sb = sb.tile([128, NT * NN], f32, name=f"o3_{name}_{e}", tag=f"st_o3{name}", bufs=2)
                    for st in range(NT):
                        nc.vector.tensor_tensor(
                            o3sb[:, st * NN:(st + 1) * NN],
                            o3pre[:, o3off + st * NN: o3off + (st + 1) * NN],
                            b3[:], Alu.add)
                    nc.sync.dma_start(od[e].rearrange("(st p) c -> p st c", p=128), o3sb[:])
                    ex3 = sb.tile([128, NT * NN], f32, name=f"ex3_{name}_{e}", tag="st_ex3", bufs=2)
                    nc.scalar.activation(ex3[:], o3sb[:], Act.Exp)
                    ex3v = ex3.rearrange("p (st c) -> p st c", c=NN)
                    s3 = sb.tile([128, NT], f32, name=f"s3_{name}_{e}", tag="st_s3", bufs=2)
                    nc.vector.reduce_sum(s3[:], ex3v, axis=Ax.X)
                    s12 = sb.tile([128, NT], f32, name=f"s12_{name}_{e}", tag="st_s12", bufs=2)
                    nc.vector.reduce_sum(s12[:], ex3v[:, :, 1:NN], axis=Ax.X)
                    r3 = sb.tile([128, NT], f32, name=f"r3_{name}_{e}", tag="st_r3", bufs=2)
                    nc.vector.reciprocal(r3[:], s3[:])
                    pv = sb.tile([128, NT], f32, name=f"p_{name}_{e}", tag=f"st_p{name}", bufs=2)
                    nc.vector.tensor_tensor(pv[:], s12[:], r3[:], Alu.mult)
                    pvec[name] = pv

                # =============== P1: GT = W.T @ h_aeT ; A = GT.T @ h_opT ========
                wbil = []
                for k in range(KD):
                    t = sb.tile([128, D2], f32r, name=f"wbil_{e}_{k}", tag="tagW", bufs=12)
                    nc.sync.dma_start(t[:], wbil_d[k * 128:(k + 1) * 128, :])
                    wbil.append(t)
                GT = []
                for et in range(KD):
                    mm = ps.tile([128, S], f32, name=f"gt_{e}_{et}", tag="psMM", bufs=2)
                    for o, w in CH:
                        for k in range(KD):
                            nc.tensor.matmul(
                                mm[:, o:o + w],
                                wbil[k][:, et * 128:(et + 1) * 128],
                                haeT[k][:, o:o + w],
                                start=(k == 0), stop=(k == KD - 1))
                    g = sb.tile([128, S], f32r, name=f"GT_{e}_{et}", tag="tagX", bufs=8)
                    nc.scalar.copy(g[:], mm[:])
                    GT.append(g)
                A = []
                for st in range(NT):
                    mm = ps.tile([128, S], f32, name=f"a_{e}_{st}", tag="psMM", bufs=2)
                    for o, w in CH:
                        for k in range(KD):
                            nc.tensor.matmul(
                                mm[:, o:o + w],
                                GT[k][:, st * 128:(st + 1) * 128],
                                hopT[k][:, o:o + w],
                                start=(k == 0), stop=(k == KD - 1))
                    a = sb.tile([128, S], f32r, name=f"A_{e}_{st}", tag="tagA", bufs=8)
                    nc.vector.tensor_copy(a[:], mm[:])
                    A.append(a)

                # =============== P2/P3: the two softmax+value paths =============
                def path(name, src_tiles, transposed, pv, hT_src, hx_tag, part):
                    """src_tiles: A tiles; if transposed, build AT via PE transpose.
                    hT_src: h_opT (op path) or h_aeT (ae path) — both the value
                    matrix (transposed) and the diag correction."""
                    E = []  # exp tiles, layout [row-tile, col 0..S)
                    for rt in range(NT):
                        if transposed:
                            src = ps.tile([128, S], f32r, name=f"at_{name}_{e}_{rt}", tag="psMM", bufs=2)
                            for st in range(NT):
                                nc.tensor.transpose(
                                    src[:, st * 128:(st + 1) * 128],
                                    src_tiles[st][:, rt * 128:(rt + 1) * 128],
                                    ident[:])
                        else:
                            src = src_tiles[rt]
                        z = sb.tile([128, S], f32, name=f"z_{name}_{e}_{rt}", tag="tagTMP", bufs=3)
                        nc.vector.scalar_tensor_tensor(
                            z[:], src[:], pv[:, rt:rt + 1], dslice(rt),
                            Alu.mult, Alu.mult)
                        f = sb.tile([128, S], f32r, name=f"E_{name}_{e}_{rt}", tag="tagW", bufs=12)
                        nc.scalar.activation(f[:], z[:], Act.Exp)
                        E.append(f)
                    # denominator: colsum(E) - 1   (diag exp(0)=1 correction)
                    csps = ps.tile([1, S], f32, name=f"cs_{name}_{e}", tag="psMM", bufs=2)
                    for o, w in CH:
                        for rt in range(NT):
                            nc.tensor.matmul(
                                csps[0:1, o:o + w],
                                ones[:, 0:1], E[rt][:, o:o + w].bitcast(f32),
                                start=(rt == 0), stop=(rt == NT - 1))
                    cs = sb.tile([1, S], f32, name=f"csb_{name}_{e}", tag="tagTMP", bufs=3)
                    nc.vector.tensor_scalar_add(cs[0:1, :], csps[0:1, :], -1.0)
                    bc = ps.tile([128, S], f32, name=f"bc_{name}_{e}", tag="psMM", bufs=2)
                    for o, w in CH:
                        nc.tensor.matmul(bc[:, o:o + w],
                                         ones[0:1, :], cs[0:1, o:o + w],
                                         start=True, stop=True)
                    rbc = sb.tile([128, S], f32, name=f"rbc_{name}_{e}", tag="tagRBC", bufs=2)
                    nc.vector.reciprocal(rbc[:], bc[:])
                    # transpose value matrix: h_x[row, d] from h_xT
                    hx = []
                    for rt in range(NT):
                        tp = ps.tile([128, D2], f32r, name=f"htp_{name}_{e}_{rt}", tag="psMM", bufs=2)
                        for dt in range(KD):
                            nc.tensor.transpose(
                                tp[:, dt * 128:(dt + 1) * 128],
                                hT_src[dt][:, rt * 128:(rt + 1) * 128],
                                ident[:])
                        hh = sb.tile([128, D2], f32r, name=f"hx_{name}_{e}_{rt}", tag=hx_tag, bufs=8)
                        nc.scalar.copy(hh[:], tp[:])
                        hx.append(hh)
                    # value matmul + diag correction + normalize + FC
                    for dt in range(KD):
                        mm = ps.tile([128, S], f32, name=f"val_{name}_{e}_{dt}", tag="psMM", bufs=2)
                        for o, w in CH:
                            for rt in range(NT):
                                nc.tensor.matmul(
                                    mm[:, o:o + w],
                                    hx[rt][:, dt * 128:(dt + 1) * 128],
                                    E[rt][:, o:o + w],
                                    start=(rt == 0), stop=(rt == NT - 1))
                        rm = sb.tile([128, S], f32, name=f"rm_{name}_{e}_{dt}", tag="tagTMP", bufs=3)
                        nc.vector.scalar_tensor_tensor(
                            rm[:], mm[:], 0.0, hT_src[dt][:], Alu.add, Alu.subtract)
                        pT = sb.tile([128, S], f32r, name=f"pT_{name}_{e}_{dt}", tag="tagAPT", bufs=2)
                        nc.vector.tensor_tensor(pT[:], rm[:], rbc[:], Alu.mult)
                        fc_mm(part, dt, pT)

                # op path first (keeps A alive), then ae path reuses slots
                path("op", A, True, pvec["op"], hopT, "tagX", 3)
                path("ae", A, False, pvec["ae"], haeT, "tagA", 1)

                # =============== final: transpose o_primeT, bias, store =========
                ofcT = sb.tile([NLP, S], f32r, name=f"ofcT_{e}", tag="st_ofcT", bufs=2)
                nc.scalar.copy(ofcT[:], fcT_ps[:])
                opsT = ps.tile([128, NT * NLP], f32r, name=f"opsT_{e}", tag="ps3", bufs=1)
                for st in range(NT):
                    nc.tensor.transpose(
                        opsT[:, st * NLP:(st + 1) * NLP],
                        ofcT[:, st * 128:(st + 1) * 128],
                        ident[0:NLP, 0:NLP])
                osb = sb.tile([128, NT * NL], f32, name=f"osb_{e}", tag="st_osb", bufs=2)
                for st in range(NT):
                    nc.vector.tensor_tensor(
                        osb[:, st * NL:(st + 1) * NL],
                        opsT[:, st * NLP: st * NLP + NL].bitcast(f32), bfc[:], Alu.add)
                nc.sync.dma_start(oprime_d[e].rearrange("(st p) c -> p st c", p=128), osb[:])

    nc.compile()
    return nc


_NC_CACHE = {}


def _get_module(n_ex=EPC, S=S_FULL):
    key = (n_ex, S)
    if key not in _NC_CACHE:
        _NC_CACHE[key] = _build_module(n_ex, S)
    return _NC_CACHE[key]


def _make_dist(S=S_FULL):
    # toeplitz: row i, col c -> delta = c - i - C0; 1/|delta| off-diag, 0 diag
    i = np.arange(128)[:, None]
    c = np.arange(C0 + S)[None, :]
    d = c - i - C0
    with np.errstate(divide="ignore"):
        m = np.where(d == 0, 0.0, 1.0 / np.abs(d).astype(np.float64)).astype(np.float32)
    return m


def _in_maps_consts(inputs, S=S_FULL):
    f = np.float32
    return {
        "wae1": np.ascontiguousarray(inputs["w_ae1"], f),
        "wop1": np.ascontiguousarray(inputs["w_op1"], f),
        "wbil": np.ascontiguousarray(inputs["W"], f),
        "wfc": np.ascontiguousarray(inputs["w_fc"], f),
        "wae3": np.ascontiguousarray(inputs["w_ae"], f),
        "wop3": np.ascontiguousarray(inputs["w_op"], f),
        "bae1": np.ascontiguousarray(np.asarray(inputs["b_ae1"], f).reshape(KH, 128).T),
        "bop1": np.ascontiguousarray(np.asarray(inputs["b_op1"], f).reshape(KH, 128).T),
        "bae3": np.ascontiguousarray(np.broadcast_to(inputs["b_ae"], (128, NN)), f),
        "bop3": np.ascontiguousarray(np.broadcast_to(inputs["b_op"], (128, NN)), f),
        "bfc": np.ascontiguousarray(np.broadcast_to(inputs["b_fc"], (128, NL)), f),
        "dist": _make_dist(S),
        "ident": np.eye(128, dtype=f),
        "ones": np.ones((128, 128), f),
    }


def _margin_loss(logits, y, margin, mask):
    N, C = logits.shape
    idx = np.arange(N)
    corr = logits[idx, y]
    hinge = np.maximum(np.float32(0.0), np.float32(margin) - corr[:, None] + logits)
    hinge[idx, y] = 0.0
    per = hinge.sum(axis=-1, dtype=np.float32) / np.float32(C)
    return (per * mask).sum(dtype=np.float32) / max(mask.sum(dtype=np.float32), np.float32(1.0))


def kernel(x, w_ae1, b_ae1, w_op1, b_op1, w_ae, b_ae, w_op, b_op, W,
           w_fc, b_fc, attention_mask, labels, labels_normal, labels_op,
           _trace=False, _trace_kwargs=None):
    from concourse.bass_utils import run_bass_kernel_spmd

    inputs = dict(w_ae1=w_ae1, b_ae1=b_ae1, w_op1=w_op1, b_op1=b_op1,
                  w_ae=w_ae, b_ae=b_ae, w_op=w_op, b_op=b_op, W=W,
                  w_fc=w_fc, b_fc=b_fc)
    nc = _get_module()
    consts = _in_maps_consts(inputs)
    xt = np.ascontiguousarray(np.asarray(x, np.float32).transpose(0, 2, 1))  # [B,H,S]
    in_maps = []
    for c in range(NCORES):
        m = dict(consts)
        m["xt"] = np.ascontiguousarray(xt[c * EPC:(c + 1) * EPC])
        in_maps.append(m)
    res = run_bass_kernel_spmd(nc, in_maps, core_ids=list(range(NCORES)),
                               trace=_trace, **(_trace_kwargs or {}))
    kernel.last_results = res
    o_prime = np.concatenate([res.results[c]["o_prime"] for c in range(NCORES)], 0)
    o_ae = np.concatenate([res.results[c]["o_ae"] for c in range(NCORES)], 0)
    o_op = np.concatenate([res.results[c]["o_op"] for c in range(NCORES)], 0)

    mask = (np.asarray(attention_mask).reshape(-1) == 1).astype(np.float32)
    loss = np.float32(0.1) * _margin_loss(o_ae.reshape(-1, NN),
                                          np.asarray(labels_normal).reshape(-1), 1.0, mask)
    loss += np.float32(0.1) * _margin_loss(o_op.reshape(-1, NN),
                                           np.asarray(labels_op).reshape(-1), 1.0, mask)
    loss += _margin_loss(o_prime.reshape(-1, NL),
                         np.asarray(labels).reshape(-1), 3.0, mask)
    return (np.float32(loss), o_prime)


# revision 20
# speedup vs baseline: 4.0190x; 4.0190x over previous
"""Trainium2 Bass kernel for nn_BILEAT_62328565400310 (sparse_attention).

Self-contained: takes FULL inputs (as from setup_inputs()), shards batch B=32
across 8 NeuronCores (4 examples/core, weights replicated), runs one fused
Bass/Tile kernel per core, gathers outputs, computes the (tiny) margin-loss
reduction on host from device-computed logits.

Device math per example (derived in transposed layouts so every matmul is a
native lhsT.T @ rhs with the contraction on partitions; float32r = fp32 data
processed at FP22 by the PE at bf16 speed):
  h_aeT[d,s], h_opT[d,s]  = w.T @ x.T           (+ per-partition bias)
  o_ae[s,3]   = relu(h_aeT).T @ w_ae            (3-class softmax -> p_ae[s])
  GT[e,s]     = W.T-as-lhsT @ h_aeT
  A[s,t]      = GT.T @ h_opT                    (bilinear scores)
  F[s,t]      = exp((A*p_ae[s]) * dist[s,t])    (dist has zero diag ->
                F diag = 1, corrected in colsum & value matmul)
  den_ae[t]   = colsum(F) - 1                   (ones-matmul)
  ae_primeT   = (h_ae-as-lhsT @ F - h_aeT) * (1/den_ae broadcast)
  (op path identical on A.T via PE transpose, p_op, h_op)
  o_prime[s,7] = [h_aeT; ae_primeT; h_opT; op_primeT]-as-lhsT @ w_fc + b_fc
"""
import numpy as np

B, S_FULL, H, D2 = 32, 1024, 768, 768
NN, NL = 3, 7
NCORES = 8
EPC = B // NCORES          # examples per core
KH = H // 128              # 6 h-tiles
KD = D2 // 128             # 6 d-tiles
C0 = 896                   # toeplitz column offset (= 128*(S/128 - 1))


def _build_module(n_ex=EPC, S=S_FULL):
    import concourse.bacc as bacc
    import concourse.mybir as mybir
    import concourse.tile as tile

    f32 = mybir.dt.float32
    f32r = mybir.dt.float32r
    Alu = mybir.AluOpType
    Act = mybir.ActivationFunctionType
    Ax = mybir.AxisListType

    NT = S // 128            # s/t tiles
    CH = [(o, min(512, S - o)) for o in range(0, S, 512)]  # free-dim chunks
    W2 = C0 + S              # toeplitz buffer width (dev configs use same C0)

    nc = bacc.Bacc(None, target_bir_lowering=False)

    # ---- DRAM I/O ----
    xt_d = nc.dram_tensor("xt", [n_ex, H, S], f32r, kind="ExternalInput")
    wae1_d = nc.dram_tensor("wae1", [H, D2], f32r, kind="ExternalInput")
    wop1_d = nc.dram_tensor("wop1", [H, D2], f32r, kind="ExternalInput")
    wbil_d = nc.dram_tensor("wbil", [D2, D2], f32r, kind="ExternalInput")
    wfc_d = nc.dram_tensor("wfc", [4 * D2, NL], f32r, kind="ExternalInput")
    wae3_d = nc.dram_tensor("wae3", [D2, NN], f32, kind="ExternalInput")
    wop3_d = nc.dram_tensor("wop3", [D2, NN], f32, kind="ExternalInput")
    bae1_d = nc.dram_tensor("bae1", [128, KH], f32, kind="ExternalInput")
    bop1_d = nc.dram_tensor("bop1", [128, KH], f32, kind="ExternalInput")
    bgt_d = nc.dram_tensor("bgt", [128, KD], f32, kind="ExternalInput")
    bae3_d = nc.dram_tensor("bae3", [128, NN], f32, kind="ExternalInput")
    bop3_d = nc.dram_tensor("bop3", [128, NN], f32, kind="ExternalInput")
    bfc_d = nc.dram_tensor("bfc", [128, NL], f32, kind="ExternalInput")
    dist_d = nc.dram_tensor("dist", [128, W2], f32, kind="ExternalInput")
    ident_d = nc.dram_tensor("ident", [128, 128], f32r, kind="ExternalInput")
    ones_d = nc.dram_tensor("ones", [128, 128], f32, kind="ExternalInput")
    oprime_d = nc.dram_tensor("o_prime", [n_ex, S, NL], f32, kind="ExternalOutput")
    oae_d = nc.dram_tensor("o_ae", [n_ex, S, NN], f32, kind="ExternalOutput")
    oop_d = nc.dram_tensor("o_op", [n_ex, S, NN], f32, kind="ExternalOutput")

    with tile.TileContext(nc) as tc:
        with (
            tc.tile_pool(name="cst", bufs=1) as cst,
            tc.tile_pool(name="sb", bufs=1) as sb,
            tc.tile_pool(name="ps", bufs=1, space="PSUM") as ps,
        ):
            # ---- persistent constants ----
            dist = cst.tile([128, W2], f32, name="dist_sb")
            nc.sync.dma_start(dist[:], dist_d[:])
            ident = cst.tile([128, 128], f32r, name="ident_sb")
            nc.sync.dma_start(ident[:], ident_d[:])
            ones = cst.tile([128, 128], f32, name="ones_sb")
            nc.sync.dma_start(ones[:], ones_d[:])
            NLP = NL + 1  # pad to even for fp32r lhsT
            wfc = cst.tile([128, 4 * KD * NLP], f32r, name="wfc_sb")
            nc.vector.memset(wfc[:].bitcast(f32), 0.0)
            for kk in range(4 * KD):
                nc.sync.dma_start(wfc[:, kk * NLP: kk * NLP + NL],
                                  wfc_d[kk * 128:(kk + 1) * 128, :])
            wae3 = cst.tile([128, KD * NN], f32, name="wae3_sb")
            wop3 = cst.tile([128, KD * NN], f32, name="wop3_sb")
            for kk in range(KD):
                nc.sync.dma_start(wae3[:, kk * NN:(kk + 1) * NN],
                                  wae3_d[kk * 128:(kk + 1) * 128, :])
                nc.sync.dma_start(wop3[:, kk * NN:(kk + 1) * NN],
                                  wop3_d[kk * 128:(kk + 1) * 128, :])
            bae1 = cst.tile([128, KH], f32, name="bae1_sb")
            nc.sync.dma_start(bae1[:], bae1_d[:])
            bop1 = cst.tile([128, KH], f32, name="bop1_sb")
            nc.sync.dma_start(bop1[:], bop1_d[:])
            bgt = cst.tile([128, KD], f32, name="bgt_sb")
            nc.sync.dma_start(bgt[:], bgt_d[:])
            bae3 = cst.tile([128, NN], f32, name="bae3_sb")
            nc.sync.dma_start(bae3[:], bae3_d[:])
            bop3 = cst.tile([128, NN], f32, name="bop3_sb")
            nc.sync.dma_start(bop3[:], bop3_d[:])
            bfc = cst.tile([128, NL], f32, name="bfc_sb")
            nc.sync.dma_start(bfc[:], bfc_d[:])

            def dslice(t):
                # dist rows for s-tile t: toeplitz slice (diag term is 0)
                return dist[:, C0 - 128 * t: C0 - 128 * t + S]

            for e in range(n_ex):
                # =============== P0: load x, weights; projections ===============
                xk = []
                for k in range(KH):
                    t = sb.tile([128, S], f32r, name=f"x_{e}_{k}", tag="tagX", bufs=8)
                    nc.sync.dma_start(t[:], xt_d[e, k * 128:(k + 1) * 128, :])
                    xk.append(t)
                wae1 = []
                wop1 = []
                for k in range(KH):
                    t = sb.tile([128, D2], f32r, name=f"wae1_{e}_{k}", tag="tagW", bufs=12)
                    nc.sync.dma_start(t[:], wae1_d[k * 128:(k + 1) * 128, :])
                    wae1.append(t)
                for k in range(KH):
                    t = sb.tile([128, D2], f32r, name=f"wop1_{e}_{k}", tag="tagW", bufs=12)
                    nc.sync.dma_start(t[:], wop1_d[k * 128:(k + 1) * 128, :])
                    wop1.append(t)

                o3_ps = ps.tile([128, 2 * NT * NN * KD], f32, name=f"o3_ps_{e}", tag="ps3", bufs=1)
                o3_v = o3_ps.rearrange("p (r c k) -> p r c k", c=NN, k=KD)
                # transposed FC accumulator: o_primeT [NLP, S]; one accumulation
                # group per 512-chunk bank spanning all 4 parts x 6 dt
                fcT_ps = ps.tile([NLP, S], f32, name=f"fcT_{e}", tag="psFC", bufs=1)

                def fc_mm(part, dt, rhs_tile):
                    for o, w in CH:
                        nc.tensor.matmul(
                            fcT_ps[:, o:o + w],
                            wfc[:, (part * KD + dt) * NLP:(part * KD + dt + 1) * NLP],
                            rhs_tile[:, o:o + w],
                            start=(part == 0 and dt == 0),
                            stop=(part == 1 and dt == KD - 1))

                haeT, hopT = [], []
                for (wsb, bvec, o3off, fcoff, hlist) in (
                    (wae1, bae1, 0, 0, haeT),
                    (wop1, bop1, NT * NN, 2 * KD, hopT),
                ):
                    for dt in range(KD):
                        mm = ps.tile([128, S], f32, name=f"proj_{e}_{dt}", tag="psMM", bufs=2)
                        for o, w in CH:
                            for k in range(KH):
                                nc.tensor.matmul(
                                    mm[:, o:o + w],
                                    wsb[k][:, dt * 128:(dt + 1) * 128],
                                    xk[k][:, o:o + w],
                                    start=(k == 0), stop=(k == KH - 1))
                        hT = sb.tile([128, S], f32r, name=f"hT_{e}_{dt}", tag="tagHT", bufs=12)
                        nc.scalar.activation(hT[:], mm[:], Act.Identity,
                                             bias=bvec[:, dt:dt + 1])
                        hlist.append(hT)
                        fc_mm(0 if fcoff == 0 else 2, dt, hT)
                        # relu copy -> o_ae/o_op logits (w_ae path then w_op path)
                        rl = sb.tile([128, S], f32r, name=f"rl_{e}_{dt}", tag="tagTMP", bufs=3)
                        nc.scalar.activation(rl[:], mm[:], Act.Relu,
                                             bias=bvec[:, dt:dt + 1])
                        w3 = wae3 if o3off == 0 else wop3
                        for st in range(NT):
                            nc.tensor.matmul(
                                o3_v[:, (0 if o3off == 0 else NT) + st, :, dt],
                                rl[:, st * 128:(st + 1) * 128].bitcast(f32),
                                w3[:, dt * NN:(dt + 1) * NN],
                                start=True, stop=True)

                # ---- 3-class softmaxes -> p_ae, p_op; emit o_ae/o_op ----
                o3pre = sb.tile([128, 2 * NT * NN], f32, name=f"o3pre_{e}", tag="st_o3pre", bufs=2)
                nc.vector.reduce_sum(o3pre[:], o3_v, axis=Ax.X)
                pvec = {}
                for name, o3off, b3, od in (("ae", 0, bae3, oae_d), ("op", NT * NN, bop3, oop_d)):
                    o3sb = sb.tile([128, NT * NN], f32, name=f"o3_{name}_{e}", tag=f"st_o3{name}", bufs=2)
                    for st in range(NT):
                        nc.vector.tensor_tensor(
                            o3sb[:, st * NN:(st + 1) * NN],
                            o3pre[:, o3off + st * NN: o3off + (st + 1) * NN],
                            b3[:], Alu.add)
                    nc.sync.dma_start(od[e].rearrange("(st p) c -> p st c", p=128), o3sb[:])
                    ex3 = sb.tile([128, NT * NN], f32, name=f"ex3_{name}_{e}", tag="st_ex3", bufs=2)
                    nc.scalar.activation(ex3[:], o3sb[:], Act.Exp)
                    ex3v = ex3.rearrange("p (st c) -> p st c", c=NN)
                    s3 = sb.tile([128, NT], f32, name=f"s3_{name}_{e}", tag="st_s3", bufs=2)
                    nc.vector.reduce_sum(s3[:], ex3v, axis=Ax.X)
                    s12 = sb.tile([128, NT], f32, name=f"s12_{name}_{e}", tag="st_s12", bufs=2)
                    nc.vector.reduce_sum(s12[:], ex3v[:, :, 1:NN], axis=Ax.X)
                    r3 = sb.tile([128, NT], f32, name=f"r3_{name}_{e}", tag="st_r3", bufs=2)
                    nc.vector.reciprocal(r3[:], s3[:])
                    pv = sb.tile([128, NT], f32, name=f"p_{name}_{e}", tag=f"st_p{name}", bufs=2)
                    nc.vector.tensor_tensor(pv[:], s12[:], r3[:], Alu.mult)
                    pvec[name] = pv

                # =============== P1: GT = W.T @ h_aeT ; A = GT.T @ h_opT ========
                wbil = []
                for k in range(KD):
                    t = sb.tile([128, D2], f32r, name=f"wbil_{e}_{k}", tag="tagW", bufs=12)
                    nc.sync.dma_start(t[:], wbil_d[k * 128:(k + 1) * 128, :])
                    wbil.append(t)
                GT = []
                for et in range(KD):
                    mm = ps.tile([128, S], f32, name=f"gt_{e}_{et}", tag="psMM", bufs=2)
                    for o, w in CH:
                        for k in range(KH):
                            nc.tensor.matmul(
                                mm[:, o:o + w],
                                wbil[k][:, et * 128:(et + 1) * 128],
                                xk[k][:, o:o + w],
                                start=(k == 0), stop=(k == KH - 1))
                    g = sb.tile([128, S], f32r, name=f"GT_{e}_{et}", tag="tagW", bufs=12)
                    nc.scalar.activation(g[:], mm[:], Act.Identity, bias=bgt[:, et:et + 1])
                    GT.append(g)
                A = []
                for st in range(NT):
                    mm = ps.tile([128, S], f32, name=f"a_{e}_{st}", tag="psMM", bufs=2)
                    for o, w in CH:
                        for k in range(KD):
                            nc.tensor.matmul(
                                mm[:, o:o + w],
                                GT[k][:, st * 128:(st + 1) * 128],
                                hopT[k][:, o:o + w],
                                start=(k == 0), stop=(k == KD - 1))
                    a = sb.tile([128, S], f32r, name=f"A_{e}_{st}", tag="tagA", bufs=8)
                    nc.vector.tensor_copy(a[:], mm[:])
                    A.append(a)

                # =============== P2/P3: the two softmax+value paths =============
                def path(name, src_tiles, transposed, pv, hT_src, hx_tag, part):
                    """src_tiles: A tiles; if transposed, build AT via PE transpose.
                    hT_src: h_opT (op path) or h_aeT (ae path) — both the value
                    matrix (transposed) and the diag correction."""
                    E = []  # exp tiles, layout [row-tile, col 0..S)
                    for rt in range(NT):
                        if transposed:
                            src = ps.tile([128, S], f32r, name=f"at_{name}_{e}_{rt}", tag="psMM", bufs=2)
                            for st in range(NT):
                                nc.tensor.transpose(
                                    src[:, st * 128:(st + 1) * 128],
                                    src_tiles[st][:, rt * 128:(rt + 1) * 128],
                                    ident[:])
                        else:
                            src = src_tiles[rt]
                        z = sb.tile([128, S], f32, name=f"z_{name}_{e}_{rt}", tag="tagTMP", bufs=3)
                        nc.vector.scalar_tensor_tensor(
                            z[:], src[:], pv[:, rt:rt + 1], dslice(rt),
                            Alu.mult, Alu.mult)
                        f = sb.tile([128, S], f32r, name=f"E_{name}_{e}_{rt}", tag="tagW", bufs=12)
                        nc.scalar.activation(f[:], z[:], Act.Exp)
                        E.append(f)
                    # denominator: colsum(E) - 1   (diag exp(0)=1 correction)
                    csps = ps.tile([1, S], f32, name=f"cs_{name}_{e}", tag="psMM", bufs=2)
                    for o, w in CH:
                        for rt in range(NT):
                            nc.tensor.matmul(
                                csps[0:1, o:o + w],
                                ones[:, 0:1], E[rt][:, o:o + w].bitcast(f32),
                                start=(rt == 0), stop=(rt == NT - 1))
                    cs = sb.tile([1, S], f32, name=f"csb_{name}_{e}", tag="tagTMP", bufs=3)
                    nc.vector.tensor_scalar_add(cs[0:1, :], csps[0:1, :], -1.0)
                    bc = ps.tile([128, S], f32, name=f"bc_{name}_{e}", tag="psMM", bufs=2)
                    for o, w in CH:
                        nc.tensor.matmul(bc[:, o:o + w],
                                         ones[0:1, :], cs[0:1, o:o + w],
                                         start=True, stop=True)
                    rbc = sb.tile([128, S], f32, name=f"rbc_{name}_{e}", tag="tagRBC", bufs=2)
                    nc.vector.reciprocal(rbc[:], bc[:])
                    # transpose value matrix: h_x[row, d] from h_xT
                    hx = []
                    for rt in range(NT):
                        tp = ps.tile([128, D2], f32r, name=f"htp_{name}_{e}_{rt}", tag="psMM", bufs=2)
                        for dt in range(KD):
                            nc.tensor.transpose(
                                tp[:, dt * 128:(dt + 1) * 128],
                                hT_src[dt][:, rt * 128:(rt + 1) * 128],
                                ident[:])
                        hh = sb.tile([128, D2], f32r, name=f"hx_{name}_{e}_{rt}", tag=hx_tag, bufs=8)
                        nc.scalar.copy(hh[:], tp[:])
                        hx.append(hh)
                    # value matmul + diag correction + normalize + FC
                    for dt in range(KD):
                        mm = ps.tile([128, S], f32, name=f"val_{name}_{e}_{dt}", tag="psMM", bufs=2)
                        for o, w in CH:
                            for rt in range(NT):
                                nc.tensor.matmul(
                                    mm[:, o:o + w],
                                    hx[rt][:, dt * 128:(dt + 1) * 128],
                                    E[rt][:, o:o + w],
                                    start=(rt == 0), stop=(rt == NT - 1))
                        rm = sb.tile([128, S], f32, name=f"rm_{name}_{e}_{dt}", tag="tagTMP", bufs=3)
                        nc.vector.scalar_tensor_tensor(
                            rm[:], mm[:], 0.0, hT_src[dt][:], Alu.add, Alu.subtract)
                        pT = sb.tile([128, S], f32r, name=f"pT_{name}_{e}_{dt}", tag="tagAPT", bufs=2)
                        nc.vector.tensor_tensor(pT[:], rm[:], rbc[:], Alu.mult)
                        fc_mm(part, dt, pT)

                # op path first (keeps A alive), then ae path reuses slots
                path("op", A, True, pvec["op"], hopT, "tagX", 3)
                path("ae", A, False, pvec["ae"], haeT, "tagA", 1)

                # =============== final: transpose o_primeT, bias, store =========
                ofcT = sb.tile([NLP, S], f32r, name=f"ofcT_{e}", tag="st_ofcT", bufs=2)
                nc.scalar.copy(ofcT[:], fcT_ps[:])
                opsT = ps.tile([128, NT * NLP], f32r, name=f"opsT_{e}", tag="ps3", bufs=1)
                for st in range(NT):
                    nc.tensor.transpose(
                        opsT[:, st * NLP:(st + 1) * NLP],
                        ofcT[:, st * 128:(st + 1) * 128],
                        ident[0:NLP, 0:NLP])
                osb = sb.tile([128, NT * NL], f32, name=f"osb_{e}", tag="st_osb", bufs=2)
                for st in range(NT):
                    nc.vector.tensor_tensor(
                        osb[:, st * NL:(st + 1) * NL],
                        opsT[:, st * NLP: st * NLP + NL].bitcast(f32), bfc[:], Alu.add)
                nc.sync.dma_start(oprime_d[e].rearrange("(st p) c -> p st c", p=128), osb[:])

    nc.compile()
    return nc


_NC_CACHE = {}


def _get_module(n_ex=EPC, S=S_FULL):
    key = (n_ex, S)
    if key not in _NC_CACHE:
        _NC_CACHE[key] = _build_module(n_ex, S)
    return _NC_CACHE[key]


def _make_dist(S=S_FULL):
    # toeplitz: row i, col c -> delta = c - i - C0; 1/|delta| off-diag, 0 diag
    i = np.arange(128)[:, None]
    c = np.arange(C0 + S)[None, :]
    d = c - i - C0
    with np.errstate(divide="ignore"):
        m = np.where(d == 0, 0.0, 1.0 / np.abs(d).astype(np.float64)).astype(np.float32)
    return m


def _in_maps_consts(inputs, S=S_FULL):
    f = np.float32
    return {
        "wae1": np.ascontiguousarray(inputs["w_ae1"], f),
        "wop1": np.ascontiguousarray(inputs["w_op1"], f),
        "wbil": np.ascontiguousarray(
            np.asarray(inputs["w_ae1"], np.float64) @ np.asarray(inputs["W"], np.float64), f),
        "bgt": np.ascontiguousarray(
            (np.asarray(inputs["b_ae1"], np.float64) @ np.asarray(inputs["W"], np.float64))
            .astype(f).reshape(KD, 128).T),
        "wfc": np.ascontiguousarray(inputs["w_fc"], f),
        "wae3": np.ascontiguousarray(inputs["w_ae"], f),
        "wop3": np.ascontiguousarray(inputs["w_op"], f),
        "bae1": np.ascontiguousarray(np.asarray(inputs["b_ae1"], f).reshape(KH, 128).T),
        "bop1": np.ascontiguousarray(np.asarray(inputs["b_op1"], f).reshape(KH, 128).T),
        "bae3": np.ascontiguousarray(np.broadcast_to(inputs["b_ae"], (128, NN)), f),
        "bop3": np.ascontiguousarray(np.broadcast_to(inputs["b_op"], (128, NN)), f),
        "bfc": np.ascontiguousarray(np.broadcast_to(inputs["b_fc"], (128, NL)), f),
        "dist": _make_dist(S),
        "ident": np.eye(128, dtype=f),
        "ones": np.ones((128, 128), f),
    }


def _margin_loss(logits, y, margin, mask):
    N, C = logits.shape
    idx = np.arange(N)
    corr = logits[idx, y]
    hinge = np.maximum(np.float32(0.0), np.float32(margin) - corr[:, None] + logits)
    hinge[idx, y] = 0.0
    per = hinge.sum(axis=-1, dtype=np.float32) / np.float32(C)
    return (per * mask).sum(dtype=np.float32) / max(mask.sum(dtype=np.float32), np.float32(1.0))


def kernel(x, w_ae1, b_ae1, w_op1, b_op1, w_ae, b_ae, w_op, b_op, W,
           w_fc, b_fc, attention_mask, labels, labels_normal, labels_op,
           _trace=False, _trace_kwargs=None):
    from concourse.bass_utils import run_bass_kernel_spmd

    inputs = dict(w_ae1=w_ae1, b_ae1=b_ae1, w_op1=w_op1, b_op1=b_op1,
                  w_ae=w_ae, b_ae=b_ae, w_op=w_op, b_op=b_op, W=W,
                  w_fc=w_fc, b_fc=b_fc)
    nc = _get_module()
    consts = _in_maps_consts(inputs)
    xt = np.ascontiguousarray(np.asarray(x, np.float32).transpose(0, 2, 1))  # [B,H,S]
    in_maps = []
    for c in range(NCORES):
        m = dict(consts)
        m["xt"] = np.ascontiguousarray(xt[c * EPC:(c + 1) * EPC])
        in_maps.append(m)
    res = run_bass_kernel_spmd(nc, in_maps, core_ids=list(range(NCORES)),
                               trace=_trace, **(_trace_kwargs or {}))
    kernel.last_results = res
    o_prime = np.concatenate([res.results[c]["o_prime"] for c in range(NCORES)], 0)
    o_ae = np.concatenate([res.results[c]["o_ae"] for c in range(NCORES)], 0)
    o_op = np.concatenate([res.results[c]["o_op"] for c in range(NCORES)], 0)

    mask = (np.asarray(attention_mask).reshape(-1) == 1).astype(np.float32)
    loss = np.float32(0.1) * _margin_loss(o_ae.reshape(-1, NN),
                                          np.asarray(labels_normal).reshape(-1), 1.0, mask)
    loss += np.float32(0.1) * _margin_loss(o_op.reshape(-1, NN),
                                           np.asarray(labels_op).reshape(-1), 1.0, mask)
    loss += _margin_loss(o_prime.reshape(-1, NL),
                         np.asarray(labels).reshape(-1), 3.0, mask)
    return (np.float32(loss), o_prime)
